# revision 25
# baseline (speedup 1.0000x reference)
"""DTM (distance-to-measure) kernel for Trainium2, 8 NeuronCores.

Math: for each grid row j, the reference sorts distances d_ji to all input
points, finds k = first index where the cumulative sorted weight reaches
wb = 0.3*sum(w), and returns sqrt((cum_wd2[k] + d2_(k)*(wb-cum_w[k]))/wb).
Writing g(tau) = sum_i w_i*min(d2_ij, tau) - tau*(W - wb), g is concave in
tau, maximized at the weighted 0.3-quantile tau*, and g(tau*) equals the
reference's dtm_val. Concavity makes the result second-order insensitive
to tau error, so the kernel only needs an approximate tau per row:

 - Host permutes the 4096 points uniformly at random, so any contiguous
   column block is a uniform sample. Count-bisection runs on the first
   SUB=1024 columns only (K1 probes from a subsample Markov bound
   hi = 3.4*mean_sub(d2)), with the last probe doubling as the secant
   anchor c(m); two more subsampled count passes at (1 +- SEC_W)*m give
   the local count slope; tau = m + (M0*SUB - c(m))*span/(ch - cl),
   clamped. Count quantile ~ weighted quantile to ~1% since weights are
   independent of distances; the secant makes the residual second order.
 - g(tau) is then evaluated exactly (per-element fp-accurate weights) in
   one scalar_tensor_tensor pass per row-tile over the bf16 d2 matrix.

Sharding: batch b = core//4, grid-row quarter q = core%4  ->  each core
handles [1024 rows x 4096 points] independently (no collectives).

d2 comes from the TensorEngine as a K=12 bf16 matmul in error-compensated
split homogeneous coordinates [Ah|Al|Ah].[Bh;Bh;Bl] with
A = [-2gx, -2gy, |g|^2, 1], B = [px, py, 1, |p|^2]  (~1e-5 relative
accuracy at full bf16 speed), evacuated to bf16 tiles h-major so the
subsample columns are ready first and bisection overlaps the remaining
matmuls. Count passes split across the Vector and Scalar engines (Sign
activation with per-partition bias; per-column targets absorb the
sign-sum transform)."""

import numpy as np
import ml_dtypes

import concourse.bacc as bacc
import concourse.mybir as mybir
from concourse import bass
from concourse.tile import TileContext
from concourse.bass_utils import run_bass_kernel_spmd

B = 2
N = 4096          # points per batch (and grid rows total)
RPC = 1024        # grid rows per core
T = RPC // 128    # 8 j-subtiles of 128 rows
M0 = 0.3

SUB = 1024        # subsample columns (uniform after host permutation)
K1 = 6            # subsample bisection probes (incl. the anchor probe)
SEC_W = 0.12      # half-width of count-slope window (rel. to anchor)
CLAMP_LO = 0.60
CLAMP_HI = 1.55
MARKOV = 3.4      # hi bracket = MARKOV * mean_sub(d2) (>= 1/M0)

N_ACT = 4         # ACT share of subsample bisection count passes
N_ACT_AS = 4      # ACT share of anchor+slope passes (must match each other
                  # per tile so sign-sum units cancel in the secant)

F32 = mybir.dt.float32
BF16 = mybir.dt.bfloat16
OP = mybir.AluOpType
AF = mybir.ActivationFunctionType


def _build_program():
    nc = bacc.Bacc()
    g12 = nc.declare_dram_parameter("g12", [12, RPC], BF16, isOutput=False)
    p12 = nc.declare_dram_parameter("p12", [12, N], BF16, isOutput=False)
    # w chunked for the PE reduce: wcols[p, c] = w[c*128 + p]
    wcols_d = nc.declare_dram_parameter("wcols", [128, N // 128], BF16,
                                        isOutput=False)
    # consts rows: 0: hi0 (d2 upper bound), 1: per-column sub count target,
    # 2: W-wb, 3: 1/wb, 4: per-column full count target
    consts = nc.declare_dram_parameter("consts", [5, T], F32, isOutput=False)
    out = nc.declare_dram_parameter("out", [128, T], F32, isOutput=True)

    def bcast(ap, parts=128):
        # replicate a [1, n] DRAM row across `parts` partitions
        return bass.AP(tensor=ap.tensor, offset=ap.offset,
                       ap=[[0, parts]] + [list(d) for d in ap.ap[1:]])

    with TileContext(nc) as tc:
        with (
            tc.tile_pool(name="persist", bufs=1) as persist,
            tc.tile_pool(name="psum", bufs=2, space="PSUM") as psum_pool,
            tc.tile_pool(name="scr", bufs=1) as scr_pool,
            tc.tile_pool(name="state", bufs=1) as state,
        ):
            # ---- load inputs ----
            g12s = persist.tile([12, RPC], BF16)
            nc.gpsimd.dma_start(out=g12s, in_=g12[:, :])
            p12s = persist.tile([12, N], BF16)
            nc.gpsimd.dma_start(out=p12s, in_=p12[:, :])

            cb = []  # broadcast const rows -> [128, T] tiles
            for r in range(5):
                t_ = persist.tile([128, T], F32, tag=f"cb{r}")
                nc.gpsimd.dma_start(out=t_, in_=bcast(consts[r:r + 1, :]))
                cb.append(t_)
            hi0_t, tgt_sub, wdiff_t, invwb_t, tgt_full = cb

            wcols = persist.tile([128, N // 128], BF16)
            nc.gpsimd.dma_start(out=wcols, in_=wcols_d[:, :])
            ones_row = persist.tile([1, 128], BF16)
            nc.vector.memset(ones_row, 1.0)

            # ---- search state ----
            lo = state.tile([128, T], F32)
            nc.vector.memset(lo, 0.0)
            step = state.tile([128, T], F32)
            mid = state.tile([128, T], F32)
            cnt = state.tile([128, T], F32)
            inv = state.tile([128, T], mybir.dt.uint8)

            d2h_t = [persist.tile([128, N], BF16, tag=f"d2h{t}", name=f"d2h{t}")
                     for t in range(T)]
            # point-major transpose of d2: d2T[p, c, j] = d2[row j, pt c*128+p]
            d2T = persist.tile([128, N // 128, RPC], BF16)
            s0acc = state.tile([128, T], F32)

            def count_pass(thr, dst, hi=SUB, n_act=N_ACT):
                # per-row count of d2h[:, :hi] <= thr; T-n_act tiles on
                # DVE, n_act on ACT (Sign + accum; targets absorb the
                # sign-sum transform)
                for t in range(T):
                    if t < T - n_act:
                        sc = scr_pool.tile([128, hi], BF16, tag="sc", bufs=2)
                        nc.vector.tensor_scalar(
                            out=sc, in0=d2h_t[t][:, :hi],
                            scalar1=thr[:, t:t + 1], scalar2=0.0,
                            op0=OP.is_le, op1=OP.add,
                            accum_out=dst[:, t:t + 1])
                    else:
                        sc = scr_pool.tile([128, hi], BF16, tag="sca", bufs=2)
                        nc.scalar.activation(
                            out=sc, in_=d2h_t[t][:, :hi], func=AF.Sign,
                            bias=thr[:, t:t + 1], scale=-1.0,
                            accum_out=dst[:, t:t + 1])

            def bisect_iter():
                nc.vector.tensor_scalar_mul(step, step, 0.5)
                nc.vector.tensor_add(out=mid, in0=lo, in1=step)
                count_pass(mid, cnt)
                # lo = mid where cnt < target (quantile above mid)
                nc.vector.tensor_tensor(
                    out=inv, in0=cnt, in1=tgt_sub, op=OP.is_lt)
                nc.vector.copy_predicated(out=lo, mask=inv, data=mid)

            # ---- phase B: bf16 d2 = G12^T P12, h-major so the h=0
            # (subsample) chunk of every row-tile lands first; h=0
            # evacuations accumulate row sums for the Markov bound.
            # Engines issue in order, so bisection iterations are EMITTED
            # between evacuation chunks to overlap with the remaining
            # matmuls (iteration k only reads d2h[:, :SUB] and state). ----
            iters_emitted = 0
            for h in range(4):
                for t in range(T):
                    pt = psum_pool.tile([128, 1024], F32, tag="mmn", bufs=3)
                    for q in range(2):
                        off = h * 1024 + q * 512
                        nc.tensor.matmul(
                            pt[:, q * 512:(q + 1) * 512],
                            g12s[:, t * 128:(t + 1) * 128],
                            p12s[:, off:off + 512],
                            start=True, stop=True,
                        )
                    dst = d2h_t[t][:, h * 1024:(h + 1) * 1024]
                    acc = s0acc[:, t:t + 1] if h == 0 else None
                    if (t + h) % 2 == 0:
                        nc.scalar.activation(
                            out=dst, in_=pt, func=AF.Copy, accum_out=acc)
                    else:
                        nc.vector.tensor_scalar(
                            out=dst, in0=pt, scalar1=1.0, scalar2=0.0,
                            op0=OP.mult, op1=OP.add, accum_out=acc)
                    # build the transposed copy while DMA queues are idle
                    nc.sync.dma_start_transpose(
                        out=d2T[:, 8 * h:8 * h + 8, t * 128:(t + 1) * 128],
                        in_=dst)
                if h == 0:
                    # Markov bound from the subsample row means
                    nc.vector.tensor_scalar_mul(step, s0acc, MARKOV / SUB)
                    nc.vector.tensor_tensor(
                        out=step, in0=step, in1=hi0_t, op=OP.min)
                else:
                    for _ in range(2):
                        if iters_emitted < K1 - 1:
                            bisect_iter()
                            iters_emitted += 1
            while iters_emitted < K1 - 1:
                bisect_iter()
                iters_emitted += 1

            # anchor m = midpoint of final bracket; the anchor probe runs
            # at FULL rate (exact count) while the slope probes at
            # (1 +- SEC_W)*m stay subsampled (slope noise is second order)
            m_t = state.tile([128, T], F32)
            nc.vector.tensor_scalar_mul(step, step, 0.5)
            nc.vector.tensor_add(out=m_t, in0=lo, in1=step)
            t1 = state.tile([128, T], F32)
            nc.vector.tensor_scalar_mul(t1, m_t, 1.0 - SEC_W)
            t2 = state.tile([128, T], F32)
            nc.vector.tensor_scalar_mul(t2, m_t, 1.0 + SEC_W)
            cl = state.tile([128, T], F32)
            ch = state.tile([128, T], F32)
            count_pass(t1, cl, n_act=N_ACT_AS)
            count_pass(t2, ch, n_act=N_ACT_AS)
            count_pass(m_t, cnt, hi=N, n_act=N_ACT_AS)

            # ---- secant: tau = m + (tgt-c)*span*(SUB/N)/(ch-cl), clamped.
            # Anchor and slope share per-tile engines, so sign-sum units
            # cancel column-wise; SUB/N converts the subsampled slope to
            # full-count units. ----
            den = state.tile([128, T], F32)
            nc.vector.tensor_sub(out=den, in0=ch, in1=cl)
            nc.vector.tensor_scalar_max(den, den, 1.0)
            rec = state.tile([128, T], F32)
            nc.vector.reciprocal(out=rec, in_=den)
            num = state.tile([128, T], F32)
            nc.vector.tensor_sub(out=num, in0=tgt_full, in1=cnt)
            nc.vector.tensor_mul(out=num, in0=num, in1=rec)
            span = state.tile([128, T], F32)
            nc.vector.tensor_sub(out=span, in0=t2, in1=t1)
            nc.vector.tensor_mul(out=num, in0=num, in1=span)
            nc.vector.tensor_scalar_mul(num, num, SUB / N)
            tau = state.tile([128, T], F32)
            nc.vector.tensor_add(out=tau, in0=m_t, in1=num)
            nc.vector.tensor_scalar_mul(t1, m_t, CLAMP_HI)
            nc.vector.tensor_tensor(out=tau, in0=tau, in1=t1, op=OP.min)
            nc.vector.tensor_scalar_mul(t1, m_t, CLAMP_LO)
            nc.vector.tensor_tensor(out=tau, in0=tau, in1=t1, op=OP.max)

            # ---- phase E: g(tau) = sum_i w_i*min(d2_i, tau) via the
            # transposed matrix: tt(min) at 2x on DVE feeds PE chunk
            # matmuls with w as the stationary vector (fp32 PSUM accum) ----
            # tau as a row vector [1, RPC] (j = t*128 + p), then replicated
            tau_row = state.tile([1, RPC], F32)
            for t in range(T):
                nc.sync.dma_start(out=tau_row[0:1, t * 128:(t + 1) * 128],
                                  in_=tau[:, t:t + 1])
            tau_row_h = state.tile([1, RPC], BF16)
            nc.vector.tensor_copy(tau_row_h, tau_row)
            # replicate across partitions via K=1 matmuls with ones
            ptb = psum_pool.tile([128, 1024], F32, tag="mmn", bufs=3)
            for q in range(2):
                nc.tensor.matmul(ptb[:, q * 512:(q + 1) * 512], ones_row,
                                 tau_row_h[0:1, q * 512:(q + 1) * 512],
                                 start=True, stop=True)
            tau_rep = persist.tile([128, RPC], BF16)
            nc.vector.tensor_copy(tau_rep, ptb)
            red = psum_pool.tile([1, RPC], F32, tag="red", bufs=1)
            NCH = N // 128
            for c in range(NCH):
                mdc = scr_pool.tile([128, RPC], BF16, tag="mdc", bufs=3)
                nc.vector.tensor_tensor(
                    out=mdc, in0=d2T[:, c, :], in1=tau_rep, op=OP.min)
                for q in range(2):
                    nc.tensor.matmul(red[:, q * 512:(q + 1) * 512],
                                     wcols[:, c:c + 1],
                                     mdc[:, q * 512:(q + 1) * 512],
                                     start=(c == 0), stop=(c == NCH - 1))
            gacc_row = state.tile([1, RPC], F32)
            nc.vector.tensor_copy(gacc_row, red)
            gacc = state.tile([128, T], F32)
            for t in range(T):
                nc.sync.dma_start(out=gacc[:, t:t + 1],
                                  in_=gacc_row[0:1, t * 128:(t + 1) * 128])

            # dtm = sqrt(max(gacc - tau*(W-wb), 0) / wb)
            tt = state.tile([128, T], F32)
            nc.vector.tensor_mul(out=tt, in0=tau, in1=wdiff_t)
            nc.vector.tensor_sub(out=tt, in0=gacc, in1=tt)
            nc.vector.tensor_mul(out=tt, in0=tt, in1=invwb_t)
            nc.vector.tensor_scalar_max(tt, tt, 0.0)
            res = state.tile([128, T], F32)
            nc.scalar.activation(out=res, in_=tt, func=AF.Sqrt)
            nc.gpsimd.dma_start(out=out[:, :], in_=res)

    nc.compile()
    return nc


def _host_prep(input, weight, grid):
    g = np.ascontiguousarray(np.asarray(grid, dtype=np.float32))
    p = np.ascontiguousarray(np.asarray(input, dtype=np.float32))
    w = np.ascontiguousarray(np.asarray(weight, dtype=np.float32))

    # fixed uniform permutation of the points: any contiguous column block
    # of the permuted matrix is a uniform sample of the 4096 points
    perm = np.random.RandomState(0xD7A).permutation(N)
    p = p[:, perm, :]
    w = w[:, perm]

    gx, gy = g[:, 0], g[:, 1]
    gn = gx * gx + gy * gy
    in_maps = []
    for core in range(8):
        b, q = divmod(core, 4)
        sl = slice(q * RPC, (q + 1) * RPC)
        g4 = np.stack([-2.0 * gx[sl], -2.0 * gy[sl], gn[sl],
                       np.ones(RPC, np.float32)]).astype(np.float32)
        px, py = p[b, :, 0], p[b, :, 1]
        pn = px * px + py * py
        p4 = np.stack([px, py, np.ones(N, np.float32), pn]).astype(np.float32)
        gh = g4.astype(ml_dtypes.bfloat16)
        gl = (g4 - gh.astype(np.float32)).astype(ml_dtypes.bfloat16)
        ph = p4.astype(ml_dtypes.bfloat16)
        pl = (p4 - ph.astype(np.float32)).astype(ml_dtypes.bfloat16)
        g12 = np.concatenate([gh, gl, gh], 0)
        p12 = np.concatenate([ph, ph, pl], 0)
        W = float(np.sum(w[b], dtype=np.float32))
        wb = np.float32(M0) * np.float32(W)
        hi0 = (np.sqrt(gn.max()) + np.sqrt(pn.max())) ** 2 * 1.0001 + 1e-6
        consts = np.empty((5, T), np.float32)
        consts[0] = hi0
        consts[1, :T - N_ACT] = M0 * SUB            # DVE count target
        consts[1, T - N_ACT:] = 2 * M0 * SUB - SUB  # ACT sign-sum target
        consts[2] = W - wb
        consts[3] = 1.0 / wb
        consts[4, :T - N_ACT_AS] = M0 * N           # full-rate anchor targets
        consts[4, T - N_ACT_AS:] = 2 * M0 * N - N
        in_maps.append({
            "g12": np.ascontiguousarray(g12),
            "p12": np.ascontiguousarray(p12),
            "wcols": np.ascontiguousarray(
                w[b].reshape(N // 128, 128).T.astype(ml_dtypes.bfloat16)),
            "consts": consts,
        })
    return in_maps


_PROGRAM = None


def kernel(input, weight, grid, _trace=False):
    global _PROGRAM
    if _PROGRAM is None:
        _PROGRAM = _build_program()
    nc = _PROGRAM
    in_maps = _host_prep(input, weight, grid)
    res = run_bass_kernel_spmd(nc, in_maps, core_ids=list(range(8)),
                               trace=_trace)
    out = np.empty((B, N), np.float32)
    for core in range(8):
        b, q = divmod(core, 4)
        # device tile [p, t] maps to row j = q*1024 + t*128 + p
        o = res.results[core]["out"]          # [128, T]
        out[b, q * RPC:(q + 1) * RPC] = o.T.reshape(-1)
    if _trace:
        kernel._last = res
    return out
